# revision 5
# baseline (speedup 1.0000x reference)
"""PhaseLinear (Catmull-Rom spline-blended 4-way linear) on 8 trn2 cores.

out[b,o] = sum_c coeff(phase[b])[c] * (input[b] @ W[c].T + bias[c])[o]

Sharding: 8 cores = 4 batch groups x 2 out_feature halves.
Core r = bi*2 + oi handles input rows [bi*256,(bi+1)*256) and
out features [oi*256,(oi+1)*256).

Per-core device kernel:
  - coeffs from phase: t = (phase - sel*0.5pi)/(1.5pi); tt=[1,t^2,t^3,0];
    c = tt @ CR_BASIS  -> computed as a cubic polynomial on DVE.
  - X^T tiles via PE transpose (identity passed from host).
  - W^T fed from host pre-transposed -> natural DMA loads (rhs).
  - 4 matmul accumulation groups (one per control point) in fp32r.
  - bias blend via K=4 matmul (coeff^T x biases).
  - final blend: 4 chained scalar_tensor_tensor ops, coeff as
    per-partition scalar.
"""

import numpy as np

import concourse.bass as bass
import concourse.tile as tile
from concourse import bacc, mybir
from concourse.bass_utils import run_bass_kernel_spmd

N_CORES = 8
B, IN_F, OUT_F, NCP = 1024, 512, 512, 4
B_SH = B // 4        # 256 batch rows per core
O_SH = OUT_F // 2    # 256 out features per core
MT = B_SH // 128     # 2 m-tiles per core
KC = IN_F // 128     # 4 k-chunks

# Catmull-Rom basis rows (only rows 0..2 used: tt = [1, t^2, t^3, 0])
CR = 0.5 * np.array(
    [[0.0, 2.0, 0.0, 0.0],
     [-1.0, 0.0, 1.0, 0.0],
     [2.0, -5.0, 4.0, -1.0],
     [-1.0, 3.0, -3.0, 1.0]], dtype=np.float64)

F32 = mybir.dt.float32
F32R = mybir.dt.float32r

_COMPILED = None


def _build():
    nc = bacc.Bacc("TRN2", target_bir_lowering=False, debug=False,
                   num_devices=N_CORES)

    x_d = nc.dram_tensor("x", [B_SH, IN_F], F32, kind="ExternalInput").ap()
    wt_d = nc.dram_tensor("wt", [NCP, IN_F, O_SH], F32R, kind="ExternalInput").ap()
    b_d = nc.dram_tensor("b", [NCP, O_SH], F32R, kind="ExternalInput").ap()
    ph_d = nc.dram_tensor("ph", [128, MT], F32, kind="ExternalInput").ap()
    id_d = nc.dram_tensor("ident", [128, 128], F32, kind="ExternalInput").ap()
    y_d = nc.dram_tensor("y", [B_SH, O_SH], F32, kind="ExternalOutput").ap()

    k_t = 1.0 / (1.5 * np.pi)           # phase -> t scale
    thresh = 1.5 * np.pi                # segment select threshold

    with tile.TileContext(nc) as tc:
        with (
            tc.tile_pool(name="const", bufs=1) as cpool,
            tc.tile_pool(name="wts", bufs=NCP) as wpool,
            tc.tile_pool(name="xin", bufs=1) as xpool,
            tc.tile_pool(name="xt", bufs=MT * KC) as xtpool,
            tc.tile_pool(name="acc", bufs=4) as apool,
            tc.tile_pool(name="tps", bufs=2, space=bass.MemorySpace.PSUM) as tpsum,
            tc.tile_pool(name="y4", bufs=2 * MT, space=bass.MemorySpace.PSUM) as ypsum,
            tc.tile_pool(name="bps", bufs=MT, space=bass.MemorySpace.PSUM) as bpsum,
        ):
            # ---- constant / small loads ----
            ident = cpool.tile([128, 128], F32)
            nc.sync.dma_start(ident[:], id_d[:])
            ph = cpool.tile([128, MT], F32)
            nc.sync.dma_start(ph[:], ph_d[:])
            bias_sb = cpool.tile([NCP, O_SH], F32R)
            nc.sync.dma_start(bias_sb[:], b_d[:])

            # ---- bulk loads ----
            # x as (128, MT, IN_F): [p, j, :] = x[j*128+p, :]
            x_sb = xpool.tile([128, MT, IN_F], F32)
            nc.sync.dma_start(
                x_sb[:], x_d.rearrange("(j p) f -> p j f", p=128))
            # weights, one DMA per control point: (128, KC, O_SH)
            wt_sb = []
            for cp in range(NCP):
                w = wpool.tile([128, KC, O_SH], F32R)
                nc.sync.dma_start(
                    w[:], wt_d[cp].rearrange("(k p) o -> p k o", p=128))
                wt_sb.append(w)

            # ---- spline coefficients on DVE, all (128, MT) ----
            m = cpool.tile([128, MT], F32)
            nc.vector.tensor_scalar(m[:], ph[:], float(thresh), None,
                                    mybir.AluOpType.is_lt)
            t1 = cpool.tile([128, MT], F32)
            nc.vector.tensor_scalar(t1[:], ph[:], float(k_t), 1.0 / 3.0,
                                    mybir.AluOpType.mult,
                                    mybir.AluOpType.subtract)
            t = cpool.tile([128, MT], F32)
            nc.vector.scalar_tensor_tensor(t[:], m[:], 1.0 / 3.0, t1[:],
                                           mybir.AluOpType.mult,
                                           mybir.AluOpType.add)
            t2 = cpool.tile([128, MT], F32)
            nc.vector.tensor_mul(t2[:], t[:], t[:])
            t3 = cpool.tile([128, MT], F32)
            nc.vector.tensor_mul(t3[:], t2[:], t[:])
            # C[:, j*NCP + cp] = coeff for (m-tile j, control point cp)
            C = cpool.tile([128, MT * NCP], F32)
            for cp in range(NCP):
                u = cpool.tile([128, MT], F32, tag="u")
                nc.vector.tensor_scalar(u[:], t2[:], float(CR[1, cp]),
                                        float(CR[0, cp]),
                                        mybir.AluOpType.mult,
                                        mybir.AluOpType.add)
                dst = C[:, cp::NCP]  # columns cp, NCP+cp, ...
                nc.vector.scalar_tensor_tensor(dst, t3[:], float(CR[2, cp]),
                                               u[:], mybir.AluOpType.mult,
                                               mybir.AluOpType.add)

            # ---- coeff^T (per m-tile) for the bias matmul ----
            ct_sb = []
            for j in range(MT):
                ct_ps = tpsum.tile([NCP, 128], F32, tag="tps")
                nc.tensor.transpose(ct_ps[:], C[:, j * NCP:(j + 1) * NCP],
                                    ident[:])
                ct = cpool.tile([NCP, 128], F32R, tag="ct")
                nc.scalar.copy(ct[:], ct_ps[:])
                ct_sb.append(ct)

            # ---- X^T tiles via PE transpose ----
            xt_sb = [[None] * KC for _ in range(MT)]
            for j in range(MT):
                for k in range(KC):
                    tp = tpsum.tile([128, 128], F32, tag="tps")
                    nc.tensor.transpose(
                        tp[:], x_sb[:, j, k * 128:(k + 1) * 128], ident[:])
                    xs = xtpool.tile([128, 128], F32R)
                    nc.scalar.copy(xs[:], tp[:])
                    xt_sb[j][k] = xs

            # ---- matmuls + blend per m-tile ----
            for j in range(MT):
                bias_ps = bpsum.tile([128, O_SH], F32)
                nc.tensor.matmul(bias_ps[:], ct_sb[j][:], bias_sb[:],
                                 start=True, stop=True)
                y4 = [ypsum.tile([128, 2 * O_SH], F32, tag="y4",
                                 name=f"y4_{j}_{h}") for h in range(2)]
                for cp in range(NCP):
                    dst = y4[cp // 2][:, (cp % 2) * O_SH:(cp % 2 + 1) * O_SH]
                    for k in range(KC):
                        nc.tensor.matmul(dst, xt_sb[j][k][:],
                                         wt_sb[cp][:, k, :],
                                         start=(k == 0), stop=(k == KC - 1))
                # blend: acc = bias + sum_cp c_cp * y4_cp
                # (DVE may read only one PSUM input per op, so bounce the
                # bias blend through SBUF on ACT first)
                bias_sbuf = apool.tile([128, O_SH], F32, tag="biascp",
                                       name=f"biascp_{j}")
                nc.scalar.copy(bias_sbuf[:], bias_ps[:])
                prev = bias_sbuf
                for cp in range(NCP):
                    nxt = apool.tile([128, O_SH], F32)
                    src = y4[cp // 2][:, (cp % 2) * O_SH:(cp % 2 + 1) * O_SH]
                    col = j * NCP + cp
                    nc.vector.scalar_tensor_tensor(
                        nxt[:], src, C[:, col:col + 1], prev[:],
                        mybir.AluOpType.mult, mybir.AluOpType.add)
                    prev = nxt
                nc.sync.dma_start(
                    y_d[j * 128:(j + 1) * 128, :], prev[:])

    nc.compile()
    return nc


def _get_compiled():
    global _COMPILED
    if _COMPILED is None:
        _COMPILED = _build()
    return _COMPILED


def _run(inputs, trace=False, tmpdir=None, trace_cores=None):
    inp = np.ascontiguousarray(inputs["input"], dtype=np.float32)
    phase = np.ascontiguousarray(inputs["phase"], dtype=np.float32)
    weights = np.ascontiguousarray(inputs["weights"], dtype=np.float32)
    biases = np.ascontiguousarray(inputs["biases"], dtype=np.float32)

    w_t = np.ascontiguousarray(weights.transpose(0, 2, 1))  # (4, IN_F, OUT_F)
    ident = np.eye(128, dtype=np.float32)

    in_maps = []
    for r in range(N_CORES):
        bi, oi = r // 2, r % 2
        ph = phase[bi * B_SH:(bi + 1) * B_SH]
        in_maps.append({
            "x": inp[bi * B_SH:(bi + 1) * B_SH],
            "wt": np.ascontiguousarray(w_t[:, :, oi * O_SH:(oi + 1) * O_SH]),
            "b": np.ascontiguousarray(biases[:, oi * O_SH:(oi + 1) * O_SH]),
            "ph": np.ascontiguousarray(ph.reshape(MT, 128).T),
            "ident": ident,
        })

    nc = _get_compiled()
    res = run_bass_kernel_spmd(nc, in_maps, list(range(N_CORES)),
                               trace=trace, tmpdir=tmpdir,
                               trace_cores=trace_cores)
    out = np.empty((B, OUT_F), dtype=np.float32)
    for r in range(N_CORES):
        bi, oi = r // 2, r % 2
        out[bi * B_SH:(bi + 1) * B_SH, oi * O_SH:(oi + 1) * O_SH] = \
            res.results[r]["y"]
    return out, res


def kernel(**inputs):
    out, _ = _run(inputs)
    return out


# revision 6
# speedup vs baseline: 1.0064x; 1.0064x over previous
"""PhaseLinear (Catmull-Rom spline-blended 4-way linear) on 8 trn2 cores.

out[b,o] = sum_c coeff(phase[b])[c] * (input[b] @ W[c].T + bias[c])[o]

Sharding: 8 cores = 4 batch groups x 2 out_feature halves.
Core r = bi*2 + oi handles input rows [bi*256,(bi+1)*256) and
out features [oi*256,(oi+1)*256).

Host-side layout prep (free): X and W are pre-transposed so the
contraction dim (in_features) lands on SBUF partitions for both matmul
operands, and control-point pairs are interleaved in the weight free
dim so one matmul streams N=512 covering two control points.

Per-core device kernel:
  - spline coeffs from phase as a cubic polynomial on DVE
    (t = (phase - sel*0.5pi)/(1.5pi); c_j = M0j + M1j t^2 + M2j t^3).
  - 4 fp32r matmul accumulation groups (m-tile x cp-pair), N=512.
  - bias blend via tiny K=4 matmul (coeff^T x biases).
  - final blend: chained scalar_tensor_tensor on DVE, coeff as
    per-partition scalar.
"""

import numpy as np

import concourse.bass as bass
import concourse.tile as tile
from concourse import bacc, mybir
from concourse.bass_utils import run_bass_kernel_spmd

N_CORES = 8
B, IN_F, OUT_F, NCP = 1024, 512, 512, 4
B_SH = B // 4        # 256 batch rows per core
O_SH = OUT_F // 2    # 256 out features per core
MT = B_SH // 128     # 2 m-tiles per core
KC = IN_F // 128     # 4 k-chunks
NG = NCP // 2        # 2 control-point pairs

# Catmull-Rom basis rows (only rows 0..2 used: tt = [1, t^2, t^3, 0])
CR = 0.5 * np.array(
    [[0.0, 2.0, 0.0, 0.0],
     [-1.0, 0.0, 1.0, 0.0],
     [2.0, -5.0, 4.0, -1.0],
     [-1.0, 3.0, -3.0, 1.0]], dtype=np.float64)

F32 = mybir.dt.float32
F32R = mybir.dt.float32r

_COMPILED = None


def _build():
    nc = bacc.Bacc("TRN2", target_bir_lowering=False, debug=False,
                   num_devices=N_CORES)

    # x^T: (IN_F, B_SH); w: (NG, IN_F, 2*O_SH) with cp pair interleaved in
    # the last dim; bias: (NCP, O_SH); ph: (128, MT); ident: (128, 128)
    xt_d = nc.dram_tensor("xt", [IN_F, B_SH], F32R, kind="ExternalInput").ap()
    wt_d = nc.dram_tensor("wt", [NG, IN_F, 2 * O_SH], F32R,
                          kind="ExternalInput").ap()
    b_d = nc.dram_tensor("b", [NCP, O_SH], F32R, kind="ExternalInput").ap()
    ph_d = nc.dram_tensor("ph", [128, MT], F32, kind="ExternalInput").ap()
    id_d = nc.dram_tensor("ident", [128, 128], F32, kind="ExternalInput").ap()
    y_d = nc.dram_tensor("y", [B_SH, O_SH], F32, kind="ExternalOutput").ap()

    k_t = 1.0 / (1.5 * np.pi)           # phase -> t scale
    thresh = 1.5 * np.pi                # segment select threshold

    with tile.TileContext(nc) as tc:
        with (
            tc.tile_pool(name="const", bufs=1) as cpool,
            tc.tile_pool(name="wts", bufs=NG) as wpool,
            tc.tile_pool(name="xt", bufs=1) as xtpool,
            tc.tile_pool(name="acc", bufs=4) as apool,
            tc.tile_pool(name="tps", bufs=2, space=bass.MemorySpace.PSUM) as tpsum,
            tc.tile_pool(name="y4", bufs=MT * NG,
                         space=bass.MemorySpace.PSUM) as ypsum,
            tc.tile_pool(name="bps", bufs=MT, space=bass.MemorySpace.PSUM) as bpsum,
        ):
            # ---- small loads ----
            ph = cpool.tile([128, MT], F32)
            nc.sync.dma_start(ph[:], ph_d[:])
            ident = cpool.tile([128, 128], F32)
            nc.sync.dma_start(ident[:], id_d[:])
            bias_sb = cpool.tile([NCP, O_SH], F32R)
            nc.sync.dma_start(bias_sb[:], b_d[:])

            # ---- bulk loads ----
            # x^T as (128, KC, B_SH): [p, k, :] = xT[k*128+p, :]
            xt_sb = xtpool.tile([128, KC, B_SH], F32R)
            nc.sync.dma_start(
                xt_sb[:], xt_d.rearrange("(k p) b -> p k b", p=128))
            # weights per cp-pair: (128, KC, 2*O_SH)
            wt_sb = []
            for g in range(NG):
                w = wpool.tile([128, KC, 2 * O_SH], F32R, tag="w",
                               name=f"w_{g}")
                nc.sync.dma_start(
                    w[:], wt_d[g].rearrange("(k p) o -> p k o", p=128))
                wt_sb.append(w)

            # ---- spline coefficients on DVE, all (128, MT) ----
            m = cpool.tile([128, MT], F32)
            nc.vector.tensor_scalar(m[:], ph[:], float(thresh), None,
                                    mybir.AluOpType.is_lt)
            t1 = cpool.tile([128, MT], F32)
            nc.vector.tensor_scalar(t1[:], ph[:], float(k_t), 1.0 / 3.0,
                                    mybir.AluOpType.mult,
                                    mybir.AluOpType.subtract)
            t = cpool.tile([128, MT], F32)
            nc.vector.scalar_tensor_tensor(t[:], m[:], 1.0 / 3.0, t1[:],
                                           mybir.AluOpType.mult,
                                           mybir.AluOpType.add)
            t2 = cpool.tile([128, MT], F32)
            nc.vector.tensor_mul(t2[:], t[:], t[:])
            t3 = cpool.tile([128, MT], F32)
            nc.vector.tensor_mul(t3[:], t2[:], t[:])
            # C[:, j*NCP + cp] = coeff for (m-tile j, control point cp)
            C = cpool.tile([128, MT * NCP], F32)
            for cp in range(NCP):
                u = cpool.tile([128, MT], F32, tag="u")
                nc.vector.tensor_scalar(u[:], t2[:], float(CR[1, cp]),
                                        float(CR[0, cp]),
                                        mybir.AluOpType.mult,
                                        mybir.AluOpType.add)
                dst = C[:, cp::NCP]  # columns cp, NCP+cp
                nc.vector.scalar_tensor_tensor(dst, t3[:], float(CR[2, cp]),
                                               u[:], mybir.AluOpType.mult,
                                               mybir.AluOpType.add)

            # ---- coeff^T (per m-tile) for the bias matmul ----
            ct_sb = []
            for j in range(MT):
                ct_ps = tpsum.tile([NCP, 128], F32, tag="tps",
                                   name=f"ctps_{j}")
                nc.tensor.transpose(ct_ps[:], C[:, j * NCP:(j + 1) * NCP],
                                    ident[:])
                ct = cpool.tile([NCP, 128], F32R, tag="ct", name=f"ct_{j}")
                nc.vector.tensor_scalar(ct[:], ct_ps[:], 1.0, None,
                                        mybir.AluOpType.mult)
                ct_sb.append(ct)

            # ---- matmuls + blend per m-tile ----
            for j in range(MT):
                bias_ps = bpsum.tile([128, O_SH], F32, tag="bps",
                                     name=f"bps_{j}")
                nc.tensor.matmul(bias_ps[:], ct_sb[j][:], bias_sb[:],
                                 start=True, stop=True)
                bias_sbuf = apool.tile([128, O_SH], F32, tag="biascp",
                                       name=f"biascp_{j}")
                nc.vector.tensor_scalar(bias_sbuf[:], bias_ps[:], 1.0, None,
                                        mybir.AluOpType.mult)
                y4 = [ypsum.tile([128, 2 * O_SH], F32, tag="y4",
                                 name=f"y4_{j}_{g}") for g in range(NG)]
                for g in range(NG):
                    for k in range(KC):
                        nc.tensor.matmul(y4[g][:],
                                         xt_sb[:, k, j * 128:(j + 1) * 128],
                                         wt_sb[g][:, k, :],
                                         start=(k == 0), stop=(k == KC - 1))
                # blend: acc = bias + sum_cp c_cp * y4_cp
                prev = bias_sbuf
                for cp in range(NCP):
                    nxt = apool.tile([128, O_SH], F32, tag="acc",
                                     name=f"acc_{j}_{cp}")
                    src = y4[cp // 2][:, (cp % 2) * O_SH:(cp % 2 + 1) * O_SH]
                    col = j * NCP + cp
                    nc.vector.scalar_tensor_tensor(
                        nxt[:], src, C[:, col:col + 1], prev[:],
                        mybir.AluOpType.mult, mybir.AluOpType.add)
                    prev = nxt
                nc.sync.dma_start(
                    y_d[j * 128:(j + 1) * 128, :], prev[:])

    nc.compile()
    return nc


def _get_compiled():
    global _COMPILED
    if _COMPILED is None:
        _COMPILED = _build()
    return _COMPILED


def _shard_inputs(inputs):
    inp = np.ascontiguousarray(inputs["input"], dtype=np.float32)
    phase = np.ascontiguousarray(inputs["phase"], dtype=np.float32)
    weights = np.ascontiguousarray(inputs["weights"], dtype=np.float32)
    biases = np.ascontiguousarray(inputs["biases"], dtype=np.float32)

    # (NG, IN_F, 2, O_SH_full...) -> per pair g, w_pair[g][i, h*OUT_F..] with
    # the two control points of the pair side by side in the last dim
    w_t = weights.transpose(0, 2, 1)  # (NCP, IN_F, OUT_F)
    ident = np.eye(128, dtype=np.float32)

    in_maps = []
    for r in range(N_CORES):
        bi, oi = r // 2, r % 2
        osl = slice(oi * O_SH, (oi + 1) * O_SH)
        x_sh = inp[bi * B_SH:(bi + 1) * B_SH]          # (B_SH, IN_F)
        ph = phase[bi * B_SH:(bi + 1) * B_SH]
        wt = np.empty((NG, IN_F, 2 * O_SH), dtype=np.float32)
        for g in range(NG):
            wt[g, :, :O_SH] = w_t[2 * g, :, osl]
            wt[g, :, O_SH:] = w_t[2 * g + 1, :, osl]
        in_maps.append({
            "xt": np.ascontiguousarray(x_sh.T),
            "wt": wt,
            "b": np.ascontiguousarray(biases[:, osl]),
            "ph": np.ascontiguousarray(ph.reshape(MT, 128).T),
            "ident": ident,
        })
    return in_maps


def _run(inputs, trace=False, tmpdir=None, trace_cores=None):
    in_maps = _shard_inputs(inputs)
    nc = _get_compiled()
    res = run_bass_kernel_spmd(nc, in_maps, list(range(N_CORES)),
                               trace=trace, tmpdir=tmpdir,
                               trace_cores=trace_cores)
    out = np.empty((B, OUT_F), dtype=np.float32)
    for r in range(N_CORES):
        bi, oi = r // 2, r % 2
        out[bi * B_SH:(bi + 1) * B_SH, oi * O_SH:(oi + 1) * O_SH] = \
            res.results[r]["y"]
    return out, res


def kernel(**inputs):
    out, _ = _run(inputs)
    return out
